# revision 43
# baseline (speedup 1.0000x reference)
"""Trainium2 Bass kernel for decode-style single-query MultiHeadAttention.

Reference computation (L=8192, E=1024, H=16, D=64):
    q = x[:1] @ Wq.T + bq                  # [1, E]
    k = x @ Wk.T + bk                      # [L, E]
    v = x @ Wv.T + bv                      # [L, E]
    per head: out_h = softmax(q_h k_h^T / sqrt(D)) v_h
    out = concat(out_h) @ Wo.T + bo        # [1, E]

Key algebraic factorization (exact, just reassociated):
    scores_h[l] = (q_h @ Wk_h) . x[l] * scale   (+ const per head -> softmax-invariant)
    attn_h @ V_h = (attn_h @ x) @ Wv_h.T + bv_h
so the device only ever contracts x against tiny [16 x E] operands
instead of materializing K/V.

Sharding: x is split along L across the 8 cores (1024 rows each); each core
splits its chunk into 2 flash blocks of 512 rows. Per block b:
    s_b = w @ x_b^T     [16, 512]
    P_b = exp(s_b)      (no max subtraction: scores are O(5), exp is f32-safe)
    d_b = rowsum(P_b)
    z_b = P_b @ x_b     [16, 1024]
Host does the tiny glue math: q/w prep, combine z_b/d_b across the 16 blocks,
V/out projections.

Precision: the z-side operands (x natural layout, P^T) are bf16. The
score-side operands ship as float8 e3m4 (4 mantissa bits), pre-scaled to
dodge e3m4's tiny exponent range (w*64, x*2; descaled inside the exp
activation's scale parameter). This keeps rel err ~1.3e-2 (< 2e-2) while
cutting the transposed-x DMA bytes in half.
"""

import os
import numpy as np
from contextlib import ExitStack

L, E, H, D = 8192, 1024, 16, 64
NCORES = 8
NL = L // NCORES  # 1024 rows of x per core
EJ = E // 128     # 8 e-chunks
LJ = NL // 128    # 8 l-chunks per core
NHALF = 2         # flash blocks per core
SCALE = 1.0 / np.sqrt(np.float32(D))

XS = 2.0          # fp8 pre-scale on x (score side)
WS = 64.0         # fp8 pre-scale on w
EXP_SCALE = 1.0 / (XS * WS)

_PROG = None
last_exec_time_ns = None
last_results = None

N_WARM = 32       # back-to-back PE warmers: pin the p-state ramp anchor


def to_bf16(a):
    import ml_dtypes

    return np.ascontiguousarray(
        np.ascontiguousarray(a, dtype=np.float32).astype(ml_dtypes.bfloat16)
    )


def to_e3m4(a):
    import ml_dtypes

    return np.ascontiguousarray(
        np.ascontiguousarray(a, dtype=np.float32).astype(ml_dtypes.float8_e3m4)
    )


def _emit(tc, tens):
    from concourse import mybir

    nc = tc.nc
    f32 = mybir.dt.float32
    bf16 = mybir.dt.bfloat16
    f8 = mybir.dt.float8e3

    with ExitStack() as ctx:
        sb = ctx.enter_context(tc.tile_pool(name="sb", bufs=1))
        pst = ctx.enter_context(tc.tile_pool(name="pst", bufs=1, space="PSUM"))
        pss = ctx.enter_context(tc.tile_pool(name="pss", bufs=1, space="PSUM"))
        psz = ctx.enter_context(tc.tile_pool(name="psz", bufs=1, space="PSUM"))

        # --- inputs ---------------------------------------------------------
        # xT e-chunk i ([128 e, LW cols]) lives at xt_all[:, i*LW:(i+1)*LW]:
        # cols 0:16 are the w^T slice for that e-chunk (riding inside the
        # same tensor, covered by the FIRST xt DMA so scores can start on
        # xt1's semaphore), cols 16:LW are x^T.
        LW = NL + H
        xt_all = sb.tile([128, EJ * LW], f8)
        # x l-chunk j ([128 l, E]) lives at x_all[:, j*E:(j+1)*E]
        x_all = sb.tile([128, LJ * E], bf16)
        id16 = sb.tile([H, H], f32)

        # scores PSUM: one tile per flash block so each block's softmax can
        # start the moment its own accumulation group finishes
        s_half = [
            pss.tile([H, 512], f32, tag=f"s{hb}", name="s_half") for hb in range(NHALF)
        ]

        # Big DMAs in stream order on the SP ring: [xt(l-half A), xt(l-half
        # B), x j=0..7]. Block A's scores+exp+z pipeline runs while the rest
        # of the stream is still in flight. Small DMAs (wt, id16) ride the
        # ACT ring so their descriptor generation interleaves with SP's.
        xtc_3d = tens["xtc"].rearrange("(i p) l -> p i l", p=128)
        xt_3d = xt_all.rearrange("p (i l) -> p i l", i=EJ)
        wt_of = lambda i: xt_all[:, i * LW: i * LW + H]
        # zero tile for the scatter-add target (block B's zout half): the
        # triggered SWDGE scatter ADDs, so its DRAM destination must be
        # zeroed first. The zero DMA rides inside the x stream (desc order
        # below keeps the head descriptor chain bubble-free).
        zero_sb = sb.tile([128, 128], f32)
        nc.gpsimd.memset(zero_sb[:], 0.0)
        # scatter idxs (identity: idx k at [k%16, k//16]) built on device:
        # iota gives c*16+p, then clamp to the 127 max row so the unused
        # upper partitions hold valid values too.
        idx_sb = sb.tile([128, 8], mybir.dt.int16)
        nc.gpsimd.iota(idx_sb[:], [[H, 8]], base=0, channel_multiplier=1)
        nc.vector.tensor_scalar_min(idx_sb[:], idx_sb[:], 127)

        nc.sync.dma_start(xt_3d[:, :, 0:512 + H], xtc_3d[:, :, 0:512 + H])
        nc.sync.dma_start(xt_3d[:, :, 512 + H:LW], xtc_3d[:, :, 512 + H:LW])
        nc.scalar.dma_start(id16[:], tens["id16"][:])
        for j in range(3):
            nc.sync.dma_start(
                x_all[:, j * E:(j + 1) * E], tens["xc"][j * 128:(j + 1) * 128, :]
            )
        for j in range(3, LJ):
            nc.sync.dma_start(
                x_all[:, j * E:(j + 1) * E], tens["xc"][j * 128:(j + 1) * 128, :]
            )
        # NOTE: Tile doesn't track WAW between DRAM writers, so there is no
        # semaphore edge from this zero-init to the triggered scatter-add
        # below. Ordering is safe by construction: the zero transfer rides
        # right after the last x chunk (~11.1us) while the scatter transfer
        # fires ~12.4us, gated on the block-B z copy chain.
        nc.sync.dma_start(tens["zout"][:, 128:256], zero_sb[:])

        # Prewarm the ACT Exp table so LoadActFuncSet happens during the DMA
        # phase instead of on the softmax critical path.
        warm = sb.tile([1, 1], f32)
        nc.gpsimd.memset(warm[:], 0.0)
        warm2 = sb.tile([1, 1], f32)
        nc.scalar.activation(warm2[:], warm[:], mybir.ActivationFunctionType.Exp)

        # PE clock-ramp warmers: harmless matmuls keep the PE activity anchor
        # pinned at t~1us so the real score matmuls run at full clock. Warm
        # against a memset tile so there is no DMA dependency.
        wz = sb.tile([128, 128], f8)
        nc.gpsimd.memset(wz[:], 0.0)
        for _ in range(N_WARM):
            nc.tensor.matmul(
                s_half[0][:, :128], wz[:, :H], wz[:, :128],
                start=True, stop=True,
            )

        # scores: s[h, l] = sum_e w[h, e] * xc[l, e] (w,x pre-scaled fp8).
        for hb in range(NHALF):
            for i in range(EJ):
                nc.tensor.matmul(
                    s_half[hb][:],
                    wt_of(i),
                    xt_all[:, i * LW + H + hb * 512: i * LW + H + (hb + 1) * 512],
                    start=(i == 0),
                    stop=(i == EJ - 1),
                )

        # exp (descaled via the activation's input scale; no max subtraction
        # needed -- true scores are O(5) so exp stays well inside f32 range)
        p_sb = sb.tile([H, NL], f32)
        pt_all = sb.tile([128, LJ * H], bf16)
        md_sb = sb.tile([H, NHALF], f32)
        # zT block hb lives at z_sb[:, hb*128:(hb+1)*128];
        # z_sb[p, hb*128 + i*16 + h] = z_block[h, i*128 + p]
        # (f32: a bf16 output path saves no time -- the <512B descriptor
        # penalty cancels the byte halving -- and costs accuracy margin)
        z_sb = sb.tile([128, NHALF * 128], f32)
        dsum, z_ps = [], []
        for hb in range(NHALF):
            ds = sb.tile([H, 1], f32, tag=f"dsum{hb}", name="dsum")
            nc.scalar.activation(
                p_sb[:, hb * 512:(hb + 1) * 512],
                s_half[hb][:],
                mybir.ActivationFunctionType.Exp,
                scale=EXP_SCALE, accum_out=ds[:],
            )
            dsum.append(ds)

        # P^T transposes + z^T matmuls, block-major so block A finishes first.
        #
        # z is computed TRANSPOSED: zT[e, h] = sum_l x[l, e] * P[h, l], i.e.
        # matmul(out=[128 e, 16 h], lhsT=x chunk [128 l, 128 e] (stationary),
        # rhs=P^T chunk [128 l, 16 h] (moving)). The moving operand is only 16
        # wide, so each of the 64 matmuls streams just 16 columns; the final
        # x chunk's contribution is 8 tiny matmuls instead of 2 wide ones,
        # which keeps the post-last-DMA-byte tail short. j-major wave order
        # so each arriving x chunk immediately retires its 8 matmuls.
        # Block B's last x chunk (j7) accumulates into its own PSUM tile so
        # that after the final DMA byte's semaphore only 8 single-matmul
        # groups remain (instead of re-running all 8 interleaved i-groups);
        # the PSUM->SBUF copy then becomes a fused add of the two tiles.
        zpb = psz.tile([128, 128], f32, tag="z1b", name="zpsb")
        for hb in range(NHALF):
            zp = psz.tile([128, 128], f32, tag=f"z{hb}", name="zps")
            z_ps.append(zp)
            # all 4 P^T chunks transpose into one PSUM tile, then a single
            # DVE copy moves them to SBUF (avoids per-chunk copy round-trips)
            ptr = pst.tile([128, 4 * H], f32, tag=f"ptr{hb}", name="ptr")
            for j in range(4 * hb, 4 * hb + 4):
                k = j - 4 * hb
                nc.tensor.transpose(
                    ptr[:, k * H:(k + 1) * H], p_sb[:, j * 128:(j + 1) * 128], id16[:]
                )
            nc.vector.tensor_copy(
                pt_all[:, 4 * hb * H:(4 * hb + 4) * H], ptr[:]
            )
            # i-major: PSUM accumulation groups within one bank cannot
            # interleave, so each i-group runs to completion. The j-gated
            # stall happens once (inside i=0); the remaining matmuls burst
            # since their x chunks have landed by then. Block B leaves j7
            # out (it lands in zpb below).
            jhi = 4 * hb + (4 if hb == 0 else 3)
            for i in range(EJ):
                for j in range(4 * hb, jhi):
                    nc.tensor.matmul(
                        zp[:, i * H:(i + 1) * H],
                        x_all[:, j * E + i * 128: j * E + (i + 1) * 128],
                        pt_all[:, j * H:(j + 1) * H],
                        start=(j == 4 * hb),
                        stop=(j == jhi - 1),
                    )
        # j7's contribution: 8 single-matmul groups (open/close sequentially
        # in one bank), the only PE work gated on the last DMA byte.
        for i in range(EJ):
            nc.tensor.matmul(
                zpb[:, i * H:(i + 1) * H],
                x_all[:, 7 * E + i * 128: 7 * E + (i + 1) * 128],
                pt_all[:, 7 * H:8 * H],
                start=True,
                stop=True,
            )
        # stage block B's j4-j6 partial into SBUF while j7 is still in
        # flight (DVE can only read one PSUM operand in the final add)
        z2a_sb = sb.tile([128, 128], f32)
        nc.vector.tensor_copy(z2a_sb[:], z_ps[1][:])

        # Block A: PSUM -> SBUF copy on ACT, then regular DMA on the ACT
        # ring (the transfer hides behind the tail of the x input stream).
        nc.vector.tensor_copy(z_sb[:, 0:128], z_ps[0][:])
        nc.scalar.dma_start(tens["zout"][:, 0:128], z_sb[:, 0:128])
        # d partials ride the idle SP ring (the ACT sequencer is busy with
        # exps; SP's descriptor gen is long done) so their completion sem
        # lands well before the scatter's and never becomes the kernel tail.
        for hb in range(NHALF):
            nc.vector.tensor_copy(md_sb[:, hb:hb + 1], dsum[hb][:])
        nc.sync.dma_start(tens["mdout"][:], md_sb[:])

        # Block B is the kernel tail: single DVE copy, then a PREPARED SWDGE
        # scatter-add fired by trigger_dma. The prep (descriptor generation,
        # ~1us on the idle Pool engine) happens mid-kernel; at the tail only
        # the transfer + completion sem remain -- this skips the 625ns HWDGE
        # descriptor generation and the 650ns DGE start delay of a regular
        # dma_start. dst row k gets src partition k (identity idxs).
        nc.vector.scalar_tensor_tensor(
            z_sb[:, 128:256], zpb[:], 1.0, z2a_sb[:],
            op0=mybir.AluOpType.mult, op1=mybir.AluOpType.add,
        )
        dma_sem = nc.alloc_semaphore("zoutb_dma")
        nc.gpsimd.dma_scatter_add(
            tens["zout"][:, 128:256],
            z_sb[:, 128:256].rearrange("p (a b) -> p a b", a=1),
            idx_sb[:],
            128,
            128,
            128,
            elem_step=NHALF * 128,
            prepare_only=True,
            sem=dma_sem,
        )
        nc.gpsimd.trigger_dma(count=None)


def _build_program():
    import concourse.tile as tile
    from concourse import bacc, mybir

    f32 = mybir.dt.float32
    bf16 = mybir.dt.bfloat16
    f8 = mybir.dt.float8e3
    nc = bacc.Bacc("TRN2", target_bir_lowering=False, debug=False, num_devices=NCORES)
    tens = {
        "xc": nc.dram_tensor("xc", [NL, E], bf16, kind="ExternalInput").ap(),
        "xtc": nc.dram_tensor("xtc", [E, NL + H], f8, kind="ExternalInput").ap(),
        "id16": nc.dram_tensor("id16", [H, H], f32, kind="ExternalInput").ap(),
        "zout": nc.dram_tensor("zout", [128, NHALF * 128], f32, kind="ExternalOutput").ap(),
        "mdout": nc.dram_tensor("mdout", [H, NHALF], f32, kind="ExternalOutput").ap(),
    }

    with tile.TileContext(nc) as tc:
        _emit(tc, tens)

    # Fixup 1: point the scatter-add prep's DMA-completion sem (on_update[0],
    # baked into the SWDGE descriptors) at the Tile-assigned DMASW lane sem
    # that the drain barrier actually waits on. The sem= argument has to be
    # supplied before the scheduler has assigned lanes, so rewire it here.
    insts = [i for b in nc.m.functions[0].blocks for i in b.instructions]
    sc = next(i for i in insts if isinstance(i, mybir.InstDMAScatterAddAnt))
    drain_w = next(
        w
        for i in insts
        if i.sync_info
        for w in i.sync_info.on_wait
        if w.ant_name and "DMASW" in str(w.ant_name)
    )
    old = sc.sync_info.on_update[0]
    sc.sync_info.on_update[0] = mybir.SyncUpdate(
        sync_type=old.sync_type,
        id=drain_w.id,
        ant_name=drain_w.ant_name,
        update_mode=old.update_mode,
        update_value=16,
        update_reg=None,
    )

    nc.compile()
    return nc


def get_prog():
    global _PROG
    if _PROG is None:
        _PROG = _build_program()
    return _PROG


def make_in_maps(x, in_proj_weight, in_proj_bias):
    """Host prep: q projection + scaled score weights, sharded x (+x^T) chunks."""
    xb = to_bf16(x)  # [L, E] bf16 for the z side
    Wq = np.asarray(in_proj_weight[:E], dtype=np.float64)
    Wk = np.asarray(in_proj_weight[E:2 * E], dtype=np.float64)
    bq = np.asarray(in_proj_bias[:E], dtype=np.float64)

    q = np.asarray(x[0:1], dtype=np.float64) @ Wq.T + bq  # [1, E]
    qh = q.reshape(H, D)                                # [16, 64]
    Wkh = Wk.reshape(H, D, E)                           # [16, 64, 1024]
    w = float(SCALE) * np.einsum("hd,hde->he", qh, Wkh)  # [16, 1024]
    # w^T rides as the FIRST 16 columns of xtc: xtc[e, h] = WS * w[h, e]
    wt_cols = to_e3m4((WS * w).astype(np.float32).T)     # [E, 16]
    id16 = np.eye(H, dtype=np.float32)
    x8 = to_e3m4(XS * np.asarray(x, dtype=np.float32))  # score-side fp8
    maps = []
    for c in range(NCORES):
        xtc = np.concatenate([wt_cols, x8[c * NL:(c + 1) * NL].T], axis=1)
        maps.append({
            "xc": np.ascontiguousarray(xb[c * NL:(c + 1) * NL]),
            "xtc": np.ascontiguousarray(xtc),
            "id16": id16,
        })
    return maps


def combine(z, d, in_proj_weight, in_proj_bias, out_proj_weight, out_proj_bias):
    """Softmax combine across partial blocks + V / out projections.

    z: [nblocks, H, E]  unnormalized P @ x per block (P = exp(s), no max)
    d: [nblocks, H]     per-block expsum
    """
    Wv = np.asarray(in_proj_weight[2 * E:], dtype=np.float64)
    bv = np.asarray(in_proj_bias[2 * E:], dtype=np.float64)

    Dn = d.astype(np.float64).sum(axis=0)               # [16]
    Z = z.astype(np.float64).sum(axis=0) / Dn[:, None]  # [16, E]

    o = np.einsum("he,hde->hd", Z, Wv.reshape(H, D, E)) + bv.reshape(H, D)  # [16, 64]
    o = o.reshape(1, E)
    out = o @ np.asarray(out_proj_weight, dtype=np.float64).T + np.asarray(
        out_proj_bias, dtype=np.float64
    )
    return out.astype(np.float32)


def run_device(in_maps, trace=False):
    from concourse import bass_utils

    global last_exec_time_ns, last_results
    nc = get_prog()
    res = bass_utils.run_bass_kernel_spmd(
        nc, in_maps, core_ids=list(range(NCORES)), trace=trace
    )
    last_exec_time_ns = res.exec_time_ns
    last_results = res
    return res


def zt_to_z(zt):
    """Device zT block [128, 128] -> z [H, E].

    zt[p, i*16 + h] = z[h, i*128 + p]
    """
    zt = np.asarray(zt, dtype=np.float64).reshape(128, EJ, H)
    return zt.transpose(2, 1, 0).reshape(H, E)


def unpack_outputs(res):
    """Device outputs -> (z [nblocks, H, E], d [nblocks, H])."""
    z, d = [], []
    for c in range(NCORES):
        zc = res.results[c]["zout"]    # [128, NHALF*128] (zT blocks)
        mc = res.results[c]["mdout"]   # [H, NHALF]
        for hb in range(NHALF):
            z.append(zt_to_z(zc[:, hb * 128:(hb + 1) * 128]))
            d.append(mc[:, hb])
    return np.stack(z), np.stack(d)


def kernel(x, in_proj_weight, in_proj_bias, out_proj_weight, out_proj_bias):
    in_maps = make_in_maps(x, in_proj_weight, in_proj_bias)
    res = run_device(in_maps, trace=os.environ.get("KERNEL_TRACE", "") == "1")
    z, d = unpack_outputs(res)
    return combine(z, d, in_proj_weight, in_proj_bias, out_proj_weight, out_proj_bias)


# revision 46
# speedup vs baseline: 1.0757x; 1.0757x over previous
"""Trainium2 Bass kernel for decode-style single-query MultiHeadAttention.

Reference computation (L=8192, E=1024, H=16, D=64):
    q = x[:1] @ Wq.T + bq                  # [1, E]
    k = x @ Wk.T + bk                      # [L, E]
    v = x @ Wv.T + bv                      # [L, E]
    per head: out_h = softmax(q_h k_h^T / sqrt(D)) v_h
    out = concat(out_h) @ Wo.T + bo        # [1, E]

Key algebraic factorization (exact, just reassociated):
    scores_h[l] = (q_h @ Wk_h) . x[l] * scale   (+ const per head -> softmax-invariant)
    attn_h @ V_h = (attn_h @ x) @ Wv_h.T + bv_h
so the device only ever contracts x against tiny [16 x E] operands
instead of materializing K/V.

Sharding: x is split along L across the 8 cores (1024 rows each); each core
splits its chunk into 2 flash blocks of 512 rows. Per block b:
    s_b = w @ x_b^T     [16, 512]
    P_b = exp(s_b)      (no max subtraction: scores are O(5), exp is f32-safe)
    d_b = rowsum(P_b)
    z_b = P_b @ x_b     [16, 1024]
Host does the tiny glue math: q/w prep, combine z_b/d_b across the 16 blocks,
V/out projections.

Precision: the z-side operands (x natural layout, P^T) are bf16. The
score-side operands ship as float8 e3m4 (4 mantissa bits), pre-scaled to
dodge e3m4's tiny exponent range (w*64, x*2; descaled inside the exp
activation's scale parameter). This keeps rel err ~1.3e-2 (< 2e-2) while
cutting the transposed-x DMA bytes in half.
"""

import os
import numpy as np
from contextlib import ExitStack

L, E, H, D = 8192, 1024, 16, 64
NCORES = 8
NL = L // NCORES  # 1024 rows of x per core
EJ = E // 128     # 8 e-chunks
LJ = NL // 128    # 8 l-chunks per core
NHALF = 2         # flash blocks per core
SCALE = 1.0 / np.sqrt(np.float32(D))

XS = 2.0          # fp8 pre-scale on x (score side)
WS = 64.0         # fp8 pre-scale on w
EXP_SCALE = 1.0 / (XS * WS)
XZ = 8.0          # fp8(e4m3) pre-scale on x (z side); z descaled by 1/XZ on host

_PROG = None
last_exec_time_ns = None
last_results = None

N_WARM = 32       # back-to-back PE warmers: pin the p-state ramp anchor


def to_bf16(a):
    import ml_dtypes

    return np.ascontiguousarray(
        np.ascontiguousarray(a, dtype=np.float32).astype(ml_dtypes.bfloat16)
    )


def to_e3m4(a):
    import ml_dtypes

    return np.ascontiguousarray(
        np.ascontiguousarray(a, dtype=np.float32).astype(ml_dtypes.float8_e3m4)
    )


def to_e4m3(a):
    import ml_dtypes

    return np.ascontiguousarray(
        np.ascontiguousarray(a, dtype=np.float32).astype(ml_dtypes.float8_e4m3)
    )


def _emit(tc, tens):
    from concourse import mybir

    nc = tc.nc
    f32 = mybir.dt.float32
    bf16 = mybir.dt.bfloat16
    f8 = mybir.dt.float8e3

    with ExitStack() as ctx:
        sb = ctx.enter_context(tc.tile_pool(name="sb", bufs=1))
        pst = ctx.enter_context(tc.tile_pool(name="pst", bufs=1, space="PSUM"))
        pss = ctx.enter_context(tc.tile_pool(name="pss", bufs=1, space="PSUM"))
        psz = ctx.enter_context(tc.tile_pool(name="psz", bufs=1, space="PSUM"))

        # --- inputs ---------------------------------------------------------
        # xT e-chunk i ([128 e, LW cols]) lives at xt_all[:, i*LW:(i+1)*LW]:
        # cols 0:16 are the w^T slice for that e-chunk (riding inside the
        # same tensor, covered by the FIRST xt DMA so scores can start on
        # xt1's semaphore), cols 16:LW are x^T.
        LW = NL + H
        xt_all = sb.tile([128, EJ * LW], f8)
        # x l-chunk j ([128 l, E]) lives at x_all[:, j*E:(j+1)*E]
        # z side also rides fp8 (e4m3, x pre-scaled by XZ; P unscaled fits
        # e4m3's 448 range) -- halves the natural-layout x stream
        x_all = sb.tile([128, LJ * E], mybir.dt.float8e4)
        id16 = sb.tile([H, H], f32)

        # scores PSUM: one tile per flash block so each block's softmax can
        # start the moment its own accumulation group finishes
        s_half = [
            pss.tile([H, 512], f32, tag=f"s{hb}", name="s_half") for hb in range(NHALF)
        ]

        # Big DMAs in stream order on the SP ring: [xt(l-half A), xt(l-half
        # B), x j=0..7]. Block A's scores+exp+z pipeline runs while the rest
        # of the stream is still in flight. Small DMAs (wt, id16) ride the
        # ACT ring so their descriptor generation interleaves with SP's.
        xtc_3d = tens["xtc"].rearrange("(i p) l -> p i l", p=128)
        xt_3d = xt_all.rearrange("p (i l) -> p i l", i=EJ)
        wt_of = lambda i: xt_all[:, i * LW: i * LW + H]
        # zero tile for the scatter-add target (block B's zout half): the
        # triggered SWDGE scatter ADDs, so its DRAM destination must be
        # zeroed first. The zero DMA rides inside the x stream (desc order
        # below keeps the head descriptor chain bubble-free).
        zero_sb = sb.tile([128, 128], f32)
        nc.gpsimd.memset(zero_sb[:], 0.0)
        # scatter idxs (identity: idx k at [k%16, k//16]) built on device:
        # iota gives c*16+p, then clamp to the 127 max row so the unused
        # upper partitions hold valid values too.
        ln2_sb = sb.tile([H, 1], f32)
        nc.gpsimd.memset(ln2_sb[:], -0.6931471805599453)
        idx_sb = sb.tile([128, 8], mybir.dt.int16)
        nc.gpsimd.iota(idx_sb[:], [[H, 8]], base=0, channel_multiplier=1)
        nc.vector.tensor_scalar_min(idx_sb[:], idx_sb[:], 127)

        nc.sync.dma_start(xt_3d[:, :, 0:512 + H], xtc_3d[:, :, 0:512 + H])
        nc.sync.dma_start(xt_3d[:, :, 512 + H:LW], xtc_3d[:, :, 512 + H:LW])
        nc.scalar.dma_start(id16[:], tens["id16"][:])
        for j in range(3):
            nc.sync.dma_start(
                x_all[:, j * E:(j + 1) * E], tens["xc"][j * 128:(j + 1) * 128, :]
            )
        for j in range(3, LJ):
            nc.sync.dma_start(
                x_all[:, j * E:(j + 1) * E], tens["xc"][j * 128:(j + 1) * 128, :]
            )
        # NOTE: Tile doesn't track WAW between DRAM writers, so there is no
        # semaphore edge from this zero-init to the triggered scatter-add
        # below. Ordering is safe by construction: the zero transfer rides
        # right after the last x chunk (~11.1us) while the scatter transfer
        # fires ~12.4us, gated on the block-B z copy chain.
        nc.sync.dma_start(tens["zout"][:, 128:256], zero_sb[:])

        # Prewarm the ACT Exp table so LoadActFuncSet happens during the DMA
        # phase instead of on the softmax critical path.
        warm = sb.tile([1, 1], f32)
        nc.gpsimd.memset(warm[:], 0.0)
        warm2 = sb.tile([1, 1], f32)
        nc.scalar.activation(warm2[:], warm[:], mybir.ActivationFunctionType.Exp)

        # PE clock-ramp warmers: harmless matmuls keep the PE activity anchor
        # pinned at t~1us so the real score matmuls run at full clock. Warm
        # against a memset tile so there is no DMA dependency.
        wz = sb.tile([128, 128], f8)
        nc.gpsimd.memset(wz[:], 0.0)
        for _ in range(N_WARM):
            nc.tensor.matmul(
                s_half[0][:, :128], wz[:, :H], wz[:, :128],
                start=True, stop=True,
            )

        # scores: s[h, l] = sum_e w[h, e] * xc[l, e] (w,x pre-scaled fp8).
        for hb in range(NHALF):
            for i in range(EJ):
                nc.tensor.matmul(
                    s_half[hb][:],
                    wt_of(i),
                    xt_all[:, i * LW + H + hb * 512: i * LW + H + (hb + 1) * 512],
                    start=(i == 0),
                    stop=(i == EJ - 1),
                )

        # exp (descaled via the activation's input scale; no max subtraction
        # needed -- true scores are O(5) so exp stays well inside f32 range)
        p_sb = sb.tile([H, NL], f32)
        pt_all = sb.tile([128, LJ * H], mybir.dt.float8e4)
        md_sb = sb.tile([H, NHALF], f32)
        # zT block hb lives at z_sb[:, hb*128:(hb+1)*128];
        # z_sb[p, hb*128 + i*16 + h] = z_block[h, i*128 + p]
        # (f32: a bf16 output path saves no time -- the <512B descriptor
        # penalty cancels the byte halving -- and costs accuracy margin)
        z_sb = sb.tile([128, NHALF * 128], f32)
        dsum, z_ps = [], []
        for hb in range(NHALF):
            ds = sb.tile([H, 1], f32, tag=f"dsum{hb}", name="dsum")
            # bias = ln(1/2) halves P so its max (~390) fits float8e4's 240
            # finite range; the softmax normalization cancels the factor
            nc.scalar.activation(
                p_sb[:, hb * 512:(hb + 1) * 512],
                s_half[hb][:],
                mybir.ActivationFunctionType.Exp,
                scale=EXP_SCALE, bias=ln2_sb[:], accum_out=ds[:],
            )
            dsum.append(ds)

        # P^T transposes + z^T matmuls, block-major so block A finishes first.
        #
        # z is computed TRANSPOSED: zT[e, h] = sum_l x[l, e] * P[h, l], i.e.
        # matmul(out=[128 e, 16 h], lhsT=x chunk [128 l, 128 e] (stationary),
        # rhs=P^T chunk [128 l, 16 h] (moving)). The moving operand is only 16
        # wide, so each of the 64 matmuls streams just 16 columns; the final
        # x chunk's contribution is 8 tiny matmuls instead of 2 wide ones,
        # which keeps the post-last-DMA-byte tail short. j-major wave order
        # so each arriving x chunk immediately retires its 8 matmuls.
        # Block B's last x chunk (j7) accumulates into its own PSUM tile so
        # that after the final DMA byte's semaphore only 8 single-matmul
        # groups remain (instead of re-running all 8 interleaved i-groups);
        # the PSUM->SBUF copy then becomes a fused add of the two tiles.
        zpb = psz.tile([128, 128], f32, tag="z1b", name="zpsb")
        for hb in range(NHALF):
            zp = psz.tile([128, 128], f32, tag=f"z{hb}", name="zps")
            z_ps.append(zp)
            # all 4 P^T chunks transpose into one PSUM tile, then a single
            # DVE copy moves them to SBUF (avoids per-chunk copy round-trips)
            ptr = pst.tile([128, 4 * H], f32, tag=f"ptr{hb}", name="ptr")
            for j in range(4 * hb, 4 * hb + 4):
                k = j - 4 * hb
                nc.tensor.transpose(
                    ptr[:, k * H:(k + 1) * H], p_sb[:, j * 128:(j + 1) * 128], id16[:]
                )
            nc.vector.tensor_copy(
                pt_all[:, 4 * hb * H:(4 * hb + 4) * H], ptr[:]
            )
            # i-major: PSUM accumulation groups within one bank cannot
            # interleave, so each i-group runs to completion. The j-gated
            # stall happens once (inside i=0); the remaining matmuls burst
            # since their x chunks have landed by then. Block B leaves j7
            # out (it lands in zpb below).
            jhi = 4 * hb + (4 if hb == 0 else 3)
            for i in range(EJ):
                for j in range(4 * hb, jhi):
                    nc.tensor.matmul(
                        zp[:, i * H:(i + 1) * H],
                        x_all[:, j * E + i * 128: j * E + (i + 1) * 128],
                        pt_all[:, j * H:(j + 1) * H],
                        start=(j == 4 * hb),
                        stop=(j == jhi - 1),
                    )
        # j7's contribution: 8 single-matmul groups (open/close sequentially
        # in one bank), the only PE work gated on the last DMA byte.
        for i in range(EJ):
            nc.tensor.matmul(
                zpb[:, i * H:(i + 1) * H],
                x_all[:, 7 * E + i * 128: 7 * E + (i + 1) * 128],
                pt_all[:, 7 * H:8 * H],
                start=True,
                stop=True,
            )
        # stage block B's j4-j6 partial into SBUF while j7 is still in
        # flight (DVE can only read one PSUM operand in the final add)
        z2a_sb = sb.tile([128, 128], f32)
        nc.vector.tensor_copy(z2a_sb[:], z_ps[1][:])

        # Block A: PSUM -> SBUF copy on ACT, then regular DMA on the ACT
        # ring (the transfer hides behind the tail of the x input stream).
        nc.vector.tensor_copy(z_sb[:, 0:128], z_ps[0][:])
        nc.scalar.dma_start(tens["zout"][:, 0:128], z_sb[:, 0:128])
        # d partials ride the idle SP ring (the ACT sequencer is busy with
        # exps; SP's descriptor gen is long done) so their completion sem
        # lands well before the scatter's and never becomes the kernel tail.
        for hb in range(NHALF):
            nc.vector.tensor_copy(md_sb[:, hb:hb + 1], dsum[hb][:])
        nc.sync.dma_start(tens["mdout"][:], md_sb[:])

        # Block B is the kernel tail: single DVE copy, then a PREPARED SWDGE
        # scatter-add fired by trigger_dma. The prep (descriptor generation,
        # ~1us on the idle Pool engine) happens mid-kernel; at the tail only
        # the transfer + completion sem remain -- this skips the 625ns HWDGE
        # descriptor generation and the 650ns DGE start delay of a regular
        # dma_start. dst row k gets src partition k (identity idxs).
        nc.vector.scalar_tensor_tensor(
            z_sb[:, 128:256], zpb[:], 1.0, z2a_sb[:],
            op0=mybir.AluOpType.mult, op1=mybir.AluOpType.add,
        )
        dma_sem = nc.alloc_semaphore("zoutb_dma")
        nc.gpsimd.dma_scatter_add(
            tens["zout"][:, 128:256],
            z_sb[:, 128:256].rearrange("p (a b) -> p a b", a=1),
            idx_sb[:],
            128,
            128,
            128,
            elem_step=NHALF * 128,
            prepare_only=True,
            sem=dma_sem,
        )
        nc.gpsimd.trigger_dma(count=None)


def _build_program():
    import concourse.tile as tile
    from concourse import bacc, mybir

    f32 = mybir.dt.float32
    bf16 = mybir.dt.bfloat16
    f8 = mybir.dt.float8e3
    nc = bacc.Bacc("TRN2", target_bir_lowering=False, debug=False, num_devices=NCORES)
    tens = {
        "xc": nc.dram_tensor("xc", [NL, E], mybir.dt.float8e4, kind="ExternalInput").ap(),
        "xtc": nc.dram_tensor("xtc", [E, NL + H], f8, kind="ExternalInput").ap(),
        "id16": nc.dram_tensor("id16", [H, H], f32, kind="ExternalInput").ap(),
        "zout": nc.dram_tensor("zout", [128, NHALF * 128], f32, kind="ExternalOutput").ap(),
        "mdout": nc.dram_tensor("mdout", [H, NHALF], f32, kind="ExternalOutput").ap(),
    }

    with tile.TileContext(nc) as tc:
        _emit(tc, tens)

    # Fixup 1: point the scatter-add prep's DMA-completion sem (on_update[0],
    # baked into the SWDGE descriptors) at the Tile-assigned DMASW lane sem
    # that the drain barrier actually waits on. The sem= argument has to be
    # supplied before the scheduler has assigned lanes, so rewire it here.
    insts = [i for b in nc.m.functions[0].blocks for i in b.instructions]
    sc = next(i for i in insts if isinstance(i, mybir.InstDMAScatterAddAnt))
    drain_w = next(
        w
        for i in insts
        if i.sync_info
        for w in i.sync_info.on_wait
        if w.ant_name and "DMASW" in str(w.ant_name)
    )
    old = sc.sync_info.on_update[0]
    sc.sync_info.on_update[0] = mybir.SyncUpdate(
        sync_type=old.sync_type,
        id=drain_w.id,
        ant_name=drain_w.ant_name,
        update_mode=old.update_mode,
        update_value=16,
        update_reg=None,
    )

    nc.compile()
    return nc


def get_prog():
    global _PROG
    if _PROG is None:
        _PROG = _build_program()
    return _PROG


def make_in_maps(x, in_proj_weight, in_proj_bias):
    """Host prep: q projection + scaled score weights, sharded x (+x^T) chunks."""
    xb = to_e4m3(XZ * np.asarray(x, dtype=np.float32))  # [L, E] e4m3 z side
    Wq = np.asarray(in_proj_weight[:E], dtype=np.float64)
    Wk = np.asarray(in_proj_weight[E:2 * E], dtype=np.float64)
    bq = np.asarray(in_proj_bias[:E], dtype=np.float64)

    q = np.asarray(x[0:1], dtype=np.float64) @ Wq.T + bq  # [1, E]
    qh = q.reshape(H, D)                                # [16, 64]
    Wkh = Wk.reshape(H, D, E)                           # [16, 64, 1024]
    w = float(SCALE) * np.einsum("hd,hde->he", qh, Wkh)  # [16, 1024]
    # w^T rides as the FIRST 16 columns of xtc: xtc[e, h] = WS * w[h, e]
    wt_cols = to_e3m4((WS * w).astype(np.float32).T)     # [E, 16]
    id16 = np.eye(H, dtype=np.float32)
    x8 = to_e3m4(XS * np.asarray(x, dtype=np.float32))  # score-side fp8
    maps = []
    for c in range(NCORES):
        xtc = np.concatenate([wt_cols, x8[c * NL:(c + 1) * NL].T], axis=1)
        maps.append({
            "xc": np.ascontiguousarray(xb[c * NL:(c + 1) * NL]),
            "xtc": np.ascontiguousarray(xtc),
            "id16": id16,
        })
    return maps


def combine(z, d, in_proj_weight, in_proj_bias, out_proj_weight, out_proj_bias):
    """Softmax combine across partial blocks + V / out projections.

    z: [nblocks, H, E]  unnormalized P @ x per block (P = exp(s), no max)
    d: [nblocks, H]     per-block expsum
    """
    Wv = np.asarray(in_proj_weight[2 * E:], dtype=np.float64)
    bv = np.asarray(in_proj_bias[2 * E:], dtype=np.float64)

    Dn = d.astype(np.float64).sum(axis=0)               # [16]
    Z = z.astype(np.float64).sum(axis=0) / XZ / Dn[:, None]  # [16, E]

    o = np.einsum("he,hde->hd", Z, Wv.reshape(H, D, E)) + bv.reshape(H, D)  # [16, 64]
    o = o.reshape(1, E)
    out = o @ np.asarray(out_proj_weight, dtype=np.float64).T + np.asarray(
        out_proj_bias, dtype=np.float64
    )
    return out.astype(np.float32)


def run_device(in_maps, trace=False):
    from concourse import bass_utils

    global last_exec_time_ns, last_results
    nc = get_prog()
    res = bass_utils.run_bass_kernel_spmd(
        nc, in_maps, core_ids=list(range(NCORES)), trace=trace
    )
    last_exec_time_ns = res.exec_time_ns
    last_results = res
    return res


def zt_to_z(zt):
    """Device zT block [128, 128] -> z [H, E].

    zt[p, i*16 + h] = z[h, i*128 + p]
    """
    zt = np.asarray(zt, dtype=np.float64).reshape(128, EJ, H)
    return zt.transpose(2, 1, 0).reshape(H, E)


def unpack_outputs(res):
    """Device outputs -> (z [nblocks, H, E], d [nblocks, H])."""
    z, d = [], []
    for c in range(NCORES):
        zc = res.results[c]["zout"]    # [128, NHALF*128] (zT blocks)
        mc = res.results[c]["mdout"]   # [H, NHALF]
        for hb in range(NHALF):
            z.append(zt_to_z(zc[:, hb * 128:(hb + 1) * 128]))
            d.append(mc[:, hb])
    return np.stack(z), np.stack(d)


def kernel(x, in_proj_weight, in_proj_bias, out_proj_weight, out_proj_bias):
    in_maps = make_in_maps(x, in_proj_weight, in_proj_bias)
    res = run_device(in_maps, trace=os.environ.get("KERNEL_TRACE", "") == "1")
    z, d = unpack_outputs(res)
    return combine(z, d, in_proj_weight, in_proj_bias, out_proj_weight, out_proj_bias)
